# revision 3
# baseline (speedup 1.0000x reference)
"""Decagon-style 2-type/4-relation GNN message passing on 8 Trainium2 cores.

Strategy (v2 — commuted aggregation + batched dma_gather):
  - SPMM commutes with the dense projection: A_k (x W_k) == (A_k x) W_k.
    So each layer aggregates RAW features first and projects after.  All
    four relations then gather from ONE shared table whose row n packs
    [h0[n] | h1[n]] (128 fp16 = 256B = the dma_gather elem granularity).
  - Nodes row-sharded 8 ways (6250/core, padded 6272).  Edges dst-sharded;
    per dst-type the edges form 4 streams (src-type j x lo/hi row band --
    the band split keeps gather indices inside int16).  Each stream is
    densely packed into 128-edge tiles (chunk-sorted, value-masked one-hot
    handles chunk boundaries mid-tile) and gathered with one dma_gather
    per 8 tiles (1024 idx) on its own SWDGE queue -- 4 queues saturate all
    4 Q7 descriptor-generation core pairs (~2.1 ns/edge vs ~8.5 serial).
  - Segment-sum via PE: psum[dst_local, :] += S_tile.T @ msg_tile where
    S[p, c] = ew[p] * (dl[p] - g*128 == c), built on DVE from resident
    int16 dl plus a repeating iota.  Per group: transpose (PE) + projection
    through the relation weights, relu, store.
  - Layer table exchange: per dst-type AllGather of the local h-slab
    (contiguous [6272, 64]) into a shared [50176, 64], then a local bounce
    copy interleaves both types into the packed [50176, 128] gather table
    (double-buffered across layers).  Layer-1 gathers read a host-built
    packed x-table (no collective, elem_step=512B selects the src type).
"""

import sys

sys.path.insert(0, "/opt/trn_rl_repo")

import numpy as np

N_NODES = 50000
F_IN = 128
H = 64
NET = 4
N_CORES = 8
P = 128
GRP = 128          # dst nodes per group == psum chunk
NS = N_NODES // N_CORES          # 6250
NS_PAD = 6272                    # 49 * 128
NG = NS_PAD // GRP               # 49
NROWS = N_CORES * NS_PAD         # 50176
SPLIT = 32768                    # int16 gather-index band boundary
CT = 8                           # tiles per dma_gather call
NI = CT * P                      # 1024 idx per call
N_LAYERS = 5
OUT_LAYERS = (0, 1, 4)


def _ceil(a, b):
    return -(-a // b)


def _prep(src, dst, ew):
    """Per dst-type edge streams.  Stream sid = 2*j + band, j = src type.

    Returns per-it dicts:
      T        total tiles (sum of padded stream tiles)
      off[s]   stream tile offset, T8[s] stream tiles (multiple of CT)
      rs       [NG][4][2] union tile span (stream-local) per (group, stream)
      need     [4][NG] running max of span end (call issue watermark)
      maxspan  max span width in tiles
      dl       [cores][P, T] int16 global dst-local (0..6271)
      ew       [cores][P, T] f16
      idx      [cores][128, T*CT] int16 wrapped+replicated gather indices
    """
    its = []
    for it in (0, 1):
        edge_data = [[None] * 4 for _ in range(N_CORES)]
        counts = np.zeros((4, N_CORES), np.int64)
        for c in range(N_CORES):
            for j in (0, 1):
                k = 2 * it + j
                m = (dst[k] >= c * NS) & (dst[k] < (c + 1) * NS)
                dl = (dst[k][m] - c * NS).astype(np.int64)
                s_ = src[k][m].astype(np.int64)
                w = ew[k][m]
                r = (s_ // NS) * NS_PAD + (s_ % NS)
                band = r >= SPLIT
                g = dl // GRP
                for b in (0, 1):
                    sid = 2 * j + b
                    mm = band == bool(b)
                    order = np.argsort(g[mm], kind="stable")
                    edge_data[c][sid] = (
                        dl[mm][order], (r[mm] - b * SPLIT)[order],
                        w[mm][order], g[mm][order],
                    )
                    counts[sid, c] = int(mm.sum())
        T8 = [_ceil(int(counts[sid].max()), P * CT) * CT for sid in range(4)]
        off = np.concatenate(([0], np.cumsum(T8))).astype(np.int64)
        T = int(off[-1])

        rs = np.zeros((NG, 4, 2), np.int64)
        rs[:, :, 0] = 1 << 40
        dl_a = np.zeros((N_CORES, P, T), np.int16)
        ew_a = np.zeros((N_CORES, P, T), np.float16)
        idx_a = np.zeros((N_CORES, 16, T * CT), np.int16)
        for c in range(N_CORES):
            for sid in range(4):
                dl_e, r_e, w_e, g_e = edge_data[c][sid]
                n = len(dl_e)
                base = int(off[sid])
                nt = T8[sid]
                gs = np.searchsorted(g_e, np.arange(NG))
                ge = np.searchsorted(g_e, np.arange(NG), side="right")
                for g in range(NG):
                    if ge[g] > gs[g]:
                        rs[g, sid, 0] = min(rs[g, sid, 0], gs[g] // P)
                        rs[g, sid, 1] = max(rs[g, sid, 1], (ge[g] - 1) // P + 1)
                vd = np.zeros(nt * P, np.int16)
                vw = np.zeros(nt * P, np.float16)
                vi = np.zeros(nt * P, np.int16)
                vd[:n] = dl_e
                vw[:n] = w_e
                vi[:n] = r_e
                dl_a[c, :, base:base + nt] = vd.reshape(nt, P).T
                ew_a[c, :, base:base + nt] = vw.reshape(nt, P).T
                # wrap per call: logical q -> [q % 16, q // 16]
                wr = (vi.reshape(-1, NI).reshape(-1, NI // 16, 16)
                      .transpose(0, 2, 1))          # [ncall, 16, NI//16]
                idx_a[c, :, base * CT:(base + nt) * CT] = (
                    np.concatenate(list(wr), axis=1))
        for g in range(NG):
            for sid in range(4):
                if rs[g, sid, 0] > rs[g, sid, 1]:   # empty
                    rs[g, sid, 0] = rs[g, sid, 1] = 0
        need = np.zeros((4, NG), np.int64)
        for sid in range(4):
            run = 0
            for g in range(NG):
                run = max(run, int(rs[g, sid, 1]))
                need[sid, g] = run
        maxspan = int((rs[:, :, 1] - rs[:, :, 0]).max())
        its.append(dict(T=T, T8=T8, off=off, rs=rs, need=need,
                        maxspan=maxspan, dl=dl_a, ew=ew_a, idx=idx_a))
    return its


def _build(prep):
    import concourse.bass as bass  # noqa: F401
    import concourse.mybir as mybir
    import concourse.tile as tile
    from concourse import bacc

    F16 = mybir.dt.float16
    F32 = mybir.dt.float32
    I16 = mybir.dt.int16
    AF = mybir.ActivationFunctionType
    OP = mybir.AluOpType

    MAXSPAN = max(prep[0]["maxspan"], prep[1]["maxspan"])

    nc = bacc.Bacc("TRN2", target_bir_lowering=False, debug=False,
                   num_devices=N_CORES, num_swdge_queues=4)

    xpk = nc.dram_tensor("xpk", [NROWS, 2 * F_IN], F16, kind="ExternalInput")
    idx_d = [nc.dram_tensor(f"idx{it}", [128, prep[it]["T"] * CT], I16,
                            kind="ExternalInput") for it in (0, 1)]
    dl_d = [nc.dram_tensor(f"dl{it}", [P, prep[it]["T"]], I16,
                           kind="ExternalInput") for it in (0, 1)]
    ew_d = [nc.dram_tensor(f"ew{it}", [P, prep[it]["T"]], F16,
                           kind="ExternalInput") for it in (0, 1)]
    w1_d = nc.dram_tensor("w1", [NET * F_IN, H], F16, kind="ExternalInput")
    wl_d = nc.dram_tensor("wl", [(N_LAYERS - 1) * 2 * P, H], F16,
                          kind="ExternalInput")
    iota_d = nc.dram_tensor("iota", [P, MAXSPAN * GRP], I16,
                            kind="ExternalInput")
    ident_d = nc.dram_tensor("ident", [P, P], F16, kind="ExternalInput")
    outs = [nc.dram_tensor(f"out{t}", [len(OUT_LAYERS) * P, NG * H], F32,
                           kind="ExternalOutput") for t in (0, 1)]

    cc = [nc.dram_tensor(f"cc{it}", [NS_PAD, H], F16) for it in (0, 1)]
    ag = [nc.dram_tensor(f"ag{it}", [NROWS, H], F16, addr_space="Shared")
          for it in (0, 1)]
    tbl = [nc.dram_tensor(f"tbl{par}", [NROWS, 2 * H], F16) for par in (0, 1)]

    with tile.TileContext(nc) as tc:
        with (
            tc.tile_pool(name="res", bufs=1) as res,
            tc.tile_pool(name="msg", bufs=4) as msgp,
            tc.tile_pool(name="sp", bufs=3) as sp,
            tc.tile_pool(name="small", bufs=3) as small,
            tc.tile_pool(name="pagg", bufs=2, space="PSUM") as pagg,
            tc.tile_pool(name="ptr", bufs=2, space="PSUM") as ptrp,
            tc.tile_pool(name="ppr", bufs=2, space="PSUM") as pprp,
        ):
            idx_sb = [res.tile([128, prep[it]["T"] * CT], I16, tag=f"ix{it}",
                               name=f"ix{it}") for it in (0, 1)]
            dl_sb = [res.tile([P, prep[it]["T"]], I16, tag=f"dl{it}",
                              name=f"dl{it}") for it in (0, 1)]
            ew_sb = [res.tile([P, prep[it]["T"]], F16, tag=f"ew{it}",
                              name=f"ew{it}") for it in (0, 1)]
            for it in (0, 1):
                nc.sync.dma_start(idx_sb[it][:], idx_d[it][:])
                nc.sync.dma_start(dl_sb[it][:], dl_d[it][:])
                nc.sync.dma_start(ew_sb[it][:], ew_d[it][:])
            iota_sb = res.tile([P, MAXSPAN * GRP], I16, tag="io", name="io")
            nc.sync.dma_start(iota_sb[:], iota_d[:])
            ident = res.tile([P, P], F16, tag="id", name="id")
            nc.sync.dma_start(ident[:], ident_d[:])
            w1_sb = res.tile([F_IN, NET * H], F16, tag="w1", name="w1")
            nc.sync.dma_start(
                w1_sb[:].rearrange("p (k f) -> p k f", k=NET),
                w1_d.ap().rearrange("(k p) f -> p k f", k=NET),
            )
            nwl = (N_LAYERS - 1) * 2
            wl_sb = res.tile([P, nwl * H], F16, tag="wl", name="wl")
            nc.sync.dma_start(
                wl_sb[:].rearrange("p (m f) -> p m f", m=nwl),
                wl_d.ap().rearrange("(m p) f -> p m f", m=nwl),
            )
            f32st = [res.tile([P, NG * H], F32, tag=f"fs{it}", name=f"fs{it}")
                     for it in (0, 1)]

            for L in range(N_LAYERS):
                par = L % 2
                for it in (0, 1):
                    pr = prep[it]
                    off, rs, need = pr["off"], pr["rs"], pr["need"]
                    # per-stream gather source AP
                    in_aps = []
                    for sid in range(4):
                        j, b = sid >> 1, sid & 1
                        if L == 0:
                            a = xpk.ap()
                            a = a[b * SPLIT:, j * F_IN:(j + 1) * F_IN]
                            in_aps.append((a, 2 * F_IN))
                        else:
                            a = tbl[par].ap()
                            a = a[b * SPLIT:, :] if b else a[:SPLIT, :]
                            in_aps.append((a, None))
                    issued = [0] * 4
                    bufs = {}

                    def issue(sid, upto_tiles, it=it, pr=pr, in_aps=in_aps,
                              issued=issued, bufs=bufs, off=off):
                        while issued[sid] * CT < upto_tiles:
                            ci = issued[sid]
                            mb = msgp.tile([P, CT * 2 * H], F16,
                                           tag=f"m{sid}", name=f"m{sid}")
                            gt = int(off[sid]) + ci * CT
                            a, step = in_aps[sid]
                            nc.gpsimd.dma_gather(
                                mb[:].rearrange("p (b e) -> p b e", e=2 * H),
                                a, idx_sb[it][:, gt * CT:(gt + CT) * CT],
                                NI, NI, 2 * H, elem_step=step,
                                queue_num=sid,
                            )
                            bufs[(sid, ci)] = mb
                            issued[sid] += 1

                    for g in range(NG):
                        gn = min(g + 1, NG - 1)
                        for sid in range(4):
                            issue(sid, int(need[sid, gn]))
                        # S segments
                        S_seg = {}
                        for sid in range(4):
                            s0, s1 = int(rs[g, sid, 0]), int(rs[g, sid, 1])
                            if s1 <= s0:
                                continue
                            span = s1 - s0
                            o = int(off[sid])
                            tdl = small.tile([P, MAXSPAN], I16, tag="td",
                                             name="td")
                            nc.vector.tensor_scalar_sub(
                                tdl[:, :span], dl_sb[it][:, o + s0:o + s1],
                                g * GRP)
                            ssb = sp.tile([P, MAXSPAN * GRP], F16,
                                          tag=f"S{sid}", name=f"S{sid}")
                            nc.vector.tensor_tensor(
                                out=ssb[:, :span * GRP].rearrange(
                                    "p (t c) -> p t c", c=GRP),
                                in0=iota_sb[:, :span * GRP].rearrange(
                                    "p (t c) -> p t c", c=GRP),
                                in1=tdl[:, :span].to_broadcast([P, span, GRP]),
                                op=OP.is_equal)
                            nc.vector.tensor_tensor(
                                out=ssb[:, :span * GRP].rearrange(
                                    "p (t c) -> p t c", c=GRP),
                                in0=ssb[:, :span * GRP].rearrange(
                                    "p (t c) -> p t c", c=GRP),
                                in1=ew_sb[it][:, o + s0:o + s1].to_broadcast(
                                    [P, span, GRP]),
                                op=OP.mult)
                            S_seg[sid] = (ssb, s0, s1)
                        # psum accumulation
                        wj = 2 * H if L == 0 else H
                        pt = pagg.tile([P, 4 * H], F32, tag="agg", name="agg",
                                       padded_shape=[P, 512])
                        for j in (0, 1):
                            chain = []
                            for b in (0, 1):
                                sid = 2 * j + b
                                if sid not in S_seg:
                                    continue
                                ssb, s0, s1 = S_seg[sid]
                                for t in range(s0, s1):
                                    chain.append((sid, ssb, s0, t))
                            for i, (sid, ssb, s0, t) in enumerate(chain):
                                ci, blk = t // CT, t % CT
                                mb = bufs[(sid, ci)]
                                if L == 0:
                                    rhs = mb[:, blk * 2 * H:(blk + 1) * 2 * H]
                                else:
                                    rhs = mb[:, blk * 2 * H + j * H:
                                             blk * 2 * H + (j + 1) * H]
                                nc.tensor.matmul(
                                    out=pt[:, j * wj:(j + 1) * wj],
                                    lhsT=ssb[:, (t - s0) * GRP:
                                             (t - s0 + 1) * GRP],
                                    rhs=rhs,
                                    start=(i == 0), stop=(i == len(chain) - 1),
                                    tile_position=(0, 0),
                                )
                        # drain: transpose + project
                        pp = pprp.tile([P, H], F32, tag="pp", name="pp",
                                       padded_shape=[P, 512])
                        if L == 0:
                            for j in (0, 1):
                                asb = small.tile([P, 2 * H], F16, tag="as",
                                                 name="as")
                                nc.scalar.activation(
                                    out=asb[:], in_=pt[:, j * 2 * H:
                                                       (j + 1) * 2 * H],
                                    func=AF.Copy)
                                ptt = ptrp.tile([P, P], F16, tag="tr",
                                                name="tr",
                                                padded_shape=[P, 1024])
                                nc.tensor.matmul(out=ptt[:], lhsT=asb[:],
                                                 rhs=ident[:],
                                                 is_transpose=True,
                                                 start=True, stop=True)
                                atr = small.tile([P, P], F16, tag="at",
                                                 name="at")
                                nc.vector.tensor_copy(out=atr[:], in_=ptt[:])
                                nc.tensor.matmul(
                                    out=pp[:],
                                    lhsT=atr[:],
                                    rhs=w1_sb[:, (2 * it + j) * H:
                                              (2 * it + j + 1) * H],
                                    start=(j == 0), stop=(j == 1),
                                )
                        else:
                            asb = small.tile([P, 2 * H], F16, tag="as",
                                             name="as")
                            nc.scalar.activation(out=asb[:], in_=pt[:, :2 * H],
                                                 func=AF.Copy)
                            ptt = ptrp.tile([P, P], F16, tag="tr", name="tr",
                                            padded_shape=[P, 1024])
                            nc.tensor.matmul(out=ptt[:], lhsT=asb[:],
                                             rhs=ident[:], is_transpose=True,
                                             start=True, stop=True)
                            atr = small.tile([P, P], F16, tag="at", name="at")
                            nc.vector.tensor_copy(out=atr[:], in_=ptt[:])
                            m = (L - 1) * 2 + it
                            nc.tensor.matmul(
                                out=pp[:], lhsT=atr[:],
                                rhs=wl_sb[:, m * H:(m + 1) * H],
                                start=True, stop=True)
                        if L in OUT_LAYERS:
                            sec = OUT_LAYERS.index(L)
                            if L < N_LAYERS - 1:
                                nc.vector.tensor_scalar_max(
                                    f32st[it][:, g * H:(g + 1) * H], pp[:],
                                    0.0)
                            else:
                                nc.vector.tensor_copy(
                                    out=f32st[it][:, g * H:(g + 1) * H],
                                    in_=pp[:])
                        if L < N_LAYERS - 1:
                            hr = small.tile([P, H], F16, tag="hr", name="hr")
                            nc.scalar.activation(out=hr[:], in_=pp[:],
                                                 func=AF.Relu)
                            nc.sync.dma_start(
                                cc[it].ap().rearrange(
                                    "(g p) h -> p g h", p=P)[:, g, :],
                                hr[:])
                    # end groups
                    if L in OUT_LAYERS:
                        sec = OUT_LAYERS.index(L)
                        nc.sync.dma_start(outs[it][sec * P:(sec + 1) * P, :],
                                          f32st[it][:])
                    if L < N_LAYERS - 1:
                        nc.gpsimd.collective_compute(
                            "AllGather", OP.bypass,
                            replica_groups=[list(range(N_CORES))],
                            ins=[cc[it].ap().opt()], outs=[ag[it].ap().opt()],
                        )
                        nc.sync.dma_start(
                            tbl[1 - par].ap()[:, it * H:(it + 1) * H],
                            ag[it].ap()[:])
    nc.compile()
    return nc


def _host_inputs(x0, x1, W1, Wl, prep):
    MAXSPAN = max(prep[0]["maxspan"], prep[1]["maxspan"])
    xpk = np.zeros((NROWS, 2 * F_IN), np.float16)
    for c in range(N_CORES):
        sl = slice(c * NS_PAD, c * NS_PAD + NS)
        xpk[sl, :F_IN] = np.asarray(x0[c * NS:(c + 1) * NS]).astype(np.float16)
        xpk[sl, F_IN:] = np.asarray(x1[c * NS:(c + 1) * NS]).astype(np.float16)
    w1 = np.asarray(W1).reshape(NET * F_IN, H).astype(np.float16)
    wl = np.zeros(((N_LAYERS - 1) * 2 * P, H), np.float16)
    Wl = np.asarray(Wl)
    for L in range(1, N_LAYERS):
        for it in (0, 1):
            blk = (L - 1) * 2 + it
            for j in (0, 1):
                wl[blk * P + j * H: blk * P + (j + 1) * H] = (
                    Wl[L - 1, 2 * it + j].astype(np.float16))
    iota = np.tile(np.arange(GRP, dtype=np.int16), MAXSPAN)[None, :].repeat(
        P, axis=0)
    ident = np.eye(P, dtype=np.float16)
    in_maps = []
    for c in range(N_CORES):
        m = {"xpk": xpk, "w1": w1, "wl": wl, "iota": iota, "ident": ident}
        for it in (0, 1):
            m[f"idx{it}"] = np.tile(prep[it]["idx"][c], (8, 1))
            m[f"dl{it}"] = prep[it]["dl"][c]
            m[f"ew{it}"] = prep[it]["ew"][c]
        in_maps.append(m)
    return in_maps


def _assemble(results):
    n_out = len(OUT_LAYERS)
    out = np.zeros((2, N_NODES, n_out * H), np.float32)
    for t in (0, 1):
        for c in range(N_CORES):
            arr = results[c][f"out{t}"]
            for s in range(n_out):
                a = (arr[s * P:(s + 1) * P]
                     .reshape(P, NG, H).transpose(1, 0, 2).reshape(NG * P, H))
                out[t, c * NS:(c + 1) * NS, s * H:(s + 1) * H] = a[:NS]
    return out


def kernel(x0, x1, src, dst, ew, W1, Wl):
    from concourse.bass_utils import run_bass_kernel_spmd

    x0 = np.asarray(x0); x1 = np.asarray(x1)
    src = np.asarray(src); dst = np.asarray(dst); ew = np.asarray(ew)

    prep = _prep(src, dst, ew)
    nc = _build(prep)
    in_maps = _host_inputs(x0, x1, W1, Wl, prep)
    global _last
    _last = (nc, in_maps)
    res = run_bass_kernel_spmd(nc, in_maps, core_ids=list(range(N_CORES)))
    return _assemble(res.results)
